# revision 17
# baseline (speedup 1.0000x reference)
"""Trainium2 Bass kernel for fused cross-adjacency:
    w = einsum('m,mtd->td', head_w, mats); z = w @ x.T + head_b
    out = where(sigmoid(z) < 0.1, 0, sigmoid(z))           # [T=64, N=100000]

Sharding: node dim N split across 8 cores (12500 nodes each); tiny params
replicated (w contracted with head_w on host - same preprocessing class as
the host-side transpose of x).

Byte-compression strategy (the f32 baseline ran at the per-core HBM
roofline, so the lever is moving fewer bytes):
  - x is shipped as fp8 e3m4 (TRN FP8_EXP3: 4 mantissa bits, range +-15.5
    covers x's +-5.2): 1 B/elem input traffic.
  - w stays bf16 (stationary operand; mixed-dtype matmul with fp8 moving).
  - output is shipped as uint8 (= round(255*adj)), dequantized on host:
    1 B/elem output traffic.
  End-to-end rel err ~8e-3 (measured vs reference), budget 2e-2.

Per-core dataflow (all buffers fit SBUF; input/sig double-buffered across
passes so nothing stalls on cross-pass reuse):
  sync   : w/bias, then the low half of x chunks (HWDGE SP ring).
  scalar : issues the high half of x chunks on the ACT HWDGE ring at pass
           start, then 4 wide sigmoid ACTIVATEs (PSUM f32 -> SBUF bf16,
           bias folded in; 2048-wide calls amortize the per-ACTIVATE
           pipeline fill).
  tensor : [64, <=512] matmuls of w.T @ x into a [128, 4096] f32 PSUM
           ring (8 banks; block i -> quarter i%4); top partition half =
           first 512-col slab, bottom half = next slab (packed rows).
  vector : t = sig*255 (tensor_scalar, bf16) then
           adj_u8 = (t >= 25.5) * t (scalar_tensor_tensor, uint8 out).
  gpsimd : 2 output DMAs per pass (u8 -> u8 SWDGE, third DMA ring), adj
           double-buffered across passes.
"""

import contextlib
import numpy as np
import ml_dtypes

import concourse.bass as bass
import concourse.mybir as mybir
from concourse.bass_utils import run_bass_kernel_spmd

N, T, D, M = 100000, 64, 128, 8
N_CORES = 8
NSH = N // N_CORES  # 12500
PACKED_W = NSH // 2  # 6250
CROSS_PRUNE = 0.1

F32 = mybir.dt.float32
BF16 = mybir.dt.bfloat16
F8E3 = mybir.dt.float8e3
U8 = mybir.dt.uint8

# Packed-output blocks (widths in packed cols; 2x that in x cols).
BLOCKS = [1024] * 6 + [106]
assert sum(BLOCKS) == PACKED_W
NB = len(BLOCKS)
BLOCK_P0 = np.concatenate([[0], np.cumsum(BLOCKS)[:-1]]).tolist()
# Block i writes PSUM quarter i % 4 (rep-local, keeps ACT call pairing).
NPSUM = 4
# previous user of block i's PSUM quarter, as (rep_delta, act_call_idx)
PSUM_PREV_CALL = {0: (1, 2), 1: (1, 2), 2: (1, 3), 3: (1, 1),
                  4: (0, 0), 5: (0, 0), 6: (0, 1)}

# ACT calls per rep: (sig packed offset, width, psum offset, blocks_done)
ACT_CALLS = [(0, 2048, 0, 2), (2048, 2048, 2048, 4),
             (4096, 2048, 0, 6), (6144, 106, 2048, 7)]
NACT = len(ACT_CALLS)

# DVE groups == ACT call regions.
DVE_GROUPS = [(0, 2048), (2048, 2048), (4096, 2048), (6144, 106)]
NDVE = len(DVE_GROUPS)

# Input DMA chunks in x cols, split across the two HWDGE rings: 'a' = sync
# (SP ring), 'b' = scalar/ACT ring.
CHUNKS_A = [(0, 4096), (4096, 2048)]
CHUNKS_B = [(6144, 4096), (10240, 2260)]
NCHA, NCHB = len(CHUNKS_A), len(CHUNKS_B)
assert sum(w for _, w in CHUNKS_A + CHUNKS_B) == NSH

NPRE = 2  # w + bias DMAs precede the chunks on the sync ring

SUBTILE = 512  # matmul moving-operand free-dim limit
N_OUT_DMA = 2  # out-DMAs per rep, split at packed col OUT_SPLIT_P
OUT_SPLIT_P = 4096
OUT_SPLIT_DVE = 2  # DVE groups covering [0, OUT_SPLIT_P)


def subtiles(width):
    """Split a packed block width into <=SUBTILE sub-tile widths."""
    out = []
    while width > 0:
        s = min(SUBTILE, width)
        out.append(s)
        width -= s
    return out


# For each block: how many chunks of each ring it needs.
def chunks_needed(i):
    xend = 2 * (BLOCK_P0[i] + BLOCKS[i])
    na = sum(1 for s, _ in CHUNKS_A if s < xend)
    nb = sum(1 for s, _ in CHUNKS_B if s < xend)
    return na, nb


# For an a-ring chunk: last block whose x range intersects it (cross-rep
# write-after-read guard in timing mode).
def last_block_touching(c):
    ce = CHUNKS_A[c][0] + CHUNKS_A[c][1]
    last = 0
    for j in range(NB):
        if 2 * BLOCK_P0[j] < ce:
            last = j
    return last


def build_nc(reps=1, probe=None):
    """reps > 1 unrolls the whole pipeline over the same data (timing: the
    per-rep slope isolates device exec time from dispatch overhead).
    probe: reduced pipelines for bottleneck isolation:
      'dma_in', 'dma_both', 'pe', 'act', 'noout'."""
    nc = bass.Bass()
    xq = nc.declare_dram_parameter("xq", [D, NSH], F8E3, isOutput=False)
    wT = nc.declare_dram_parameter("wT", [D, T], BF16, isOutput=False)
    biasd = nc.declare_dram_parameter("biasd", [D, 1], F32, isOutput=False)
    out = nc.declare_dram_parameter("out", [D, PACKED_W], U8, isOutput=True)

    ctx = contextlib.ExitStack()
    with ctx:
        xt = [
            ctx.enter_context(nc.sbuf_tensor(f"xt{p}", [D, NSH], F8E3))
            for p in range(2)
        ]
        w_sb = ctx.enter_context(nc.sbuf_tensor("w_sb", [D, T], BF16))
        bias_sb = ctx.enter_context(nc.sbuf_tensor("bias_sb", [D, 1], F32))
        sigtab = ctx.enter_context(nc.sbuf_tensor("sigtab", [D, 1], BF16))
        sig = [
            ctx.enter_context(nc.sbuf_tensor(f"sig{p}", [D, PACKED_W], BF16))
            for p in range(2)
        ]
        t255 = ctx.enter_context(nc.sbuf_tensor("t255", [D, PACKED_W], BF16))
        adj = [
            ctx.enter_context(nc.sbuf_tensor(f"adj{p}", [D, PACKED_W], U8))
            for p in range(2)
        ]
        zg = ctx.enter_context(nc.psum_tensor("zg", [D, 4096], F32))

        s_in = ctx.enter_context(nc.semaphore("s_in"))
        s_in_b = ctx.enter_context(nc.semaphore("s_in_b"))
        s_mm = ctx.enter_context(nc.semaphore("s_mm"))
        s_sig = ctx.enter_context(nc.semaphore("s_sig"))
        s_adj = ctx.enter_context(nc.semaphore("s_adj"))
        s_out = ctx.enter_context(nc.semaphore("s_out"))

        block = ctx.enter_context(nc.Block())

        @block.sync
        def _(sync):
            sync.dma_start(out=w_sb[:, :], in_=wT[:, :]).then_inc(s_in, 16)
            sync.dma_start(out=bias_sb[:, :], in_=biasd[:, :]).then_inc(s_in, 16)
            for r in range(reps):
                for c, (cs, cw) in enumerate(CHUNKS_A):
                    if r >= 2 and probe not in ('dma_in', 'dma_both'):
                        # xt[r%2] reuse: PE of rep r-2 must be done with it
                        sync.wait_ge(
                            s_mm, (r - 2) * NB + last_block_touching(c) + 1
                        )
                    sync.dma_start(
                        out=xt[r % 2][:, cs : cs + cw],
                        in_=xq[:, cs : cs + cw],
                    ).then_inc(s_in, 16)
                if probe == 'dma_in':
                    # b-ring chunks ride the sync ring too in this probe
                    for cs, cw in CHUNKS_B:
                        sync.dma_start(
                            out=xt[r % 2][:, cs : cs + cw],
                            in_=xq[:, cs : cs + cw],
                        ).then_inc(s_in_b, 16)

        @block.tensor
        def _(pe):
            if probe in ('dma_in', 'dma_both'):
                return
            pe.wait_ge(s_in, 16)  # w loaded
            for r in range(reps):
                for i, bw in enumerate(BLOCKS):
                    na, nb = chunks_needed(i)
                    pe.wait_ge(s_in, 16 * (NPRE + r * NCHA + na))
                    if nb:
                        pe.wait_ge(s_in_b, 16 * (r * NCHB + nb))
                    if probe != 'pe':
                        # PSUM quarter reuse: the ACT call that consumed its
                        # previous occupant must be done
                        dr, call = PSUM_PREV_CALL[i]
                        v = (r - dr) * NACT + call + 1
                        if v >= 1:
                            pe.wait_ge(s_sig, v)
                    q0 = 1024 * (i % NPSUM)
                    p0 = BLOCK_P0[i]
                    # top halves (partitions 0:64) then bottoms; with >=2
                    # subtiles per half, same-bank matmuls are never
                    # back-to-back so no intra-block drain is needed
                    nsub = len(subtiles(bw))
                    for half in range(2):
                        q = 0
                        for s in subtiles(bw):
                            x0 = 2 * (p0 + q) + half * s
                            pe.matmul(
                                zg[64 * half : 64 * half + 64,
                                   q0 + q : q0 + q + s],
                                w_sb[:, :],
                                xt[r % 2][:, x0 : x0 + s],
                                start=True, stop=True,
                            )
                            q += s
                        if half == 0 and nsub == 1:
                            pe.drain()
                    pe.drain().then_inc(s_mm, 1)

        @block.scalar
        def _(act):
            if probe in ('dma_in', 'pe'):
                return
            if probe == 'dma_both':
                for r in range(reps):
                    for cs, cw in CHUNKS_B:
                        act.dma_start(
                            out=xt[r % 2][:, cs : cs + cw],
                            in_=xq[:, cs : cs + cw],
                        ).then_inc(s_in_b, 16)
                return
            act.wait_ge(s_in, 32)  # bias loaded
            # preload the sigmoid table set during the fill phase
            act.activation(
                sigtab[:, 0:1], bias_sb[:, 0:1],
                mybir.ActivationFunctionType.Sigmoid, bias=bias_sb[:, 0:1],
            )
            for r in range(reps):
                # b-ring input chunks for this rep; the preceding rep's last
                # ACTIVATE already implies the overwritten xt buffer is free
                for cs, cw in CHUNKS_B:
                    act.dma_start(
                        out=xt[r % 2][:, cs : cs + cw],
                        in_=xq[:, cs : cs + cw],
                    ).then_inc(s_in_b, 16)
                for j, (p0, w, zoff, blocks_done) in enumerate(ACT_CALLS):
                    act.wait_ge(s_mm, r * NB + blocks_done)
                    if r >= 2 and probe is None:
                        # sig[r%2] reuse: DVE group of rep r-2 done
                        act.wait_ge(s_adj, (r - 2) * NDVE + j + 1)
                    act.activation(
                        sig[r % 2][:, p0 : p0 + w],
                        zg[:, zoff : zoff + w],
                        mybir.ActivationFunctionType.Sigmoid,
                        bias=bias_sb[:, 0:1],
                    ).then_inc(s_sig, 1)

        @block.vector
        def _(dve):
            if probe in ('dma_in', 'dma_both', 'pe', 'act'):
                return
            for r in range(reps):
                if r >= 2 and probe is None:
                    # adj[r%2] reuse: rep r-2's output DMAs must be done
                    dve.wait_ge(s_out, 16 * N_OUT_DMA * (r - 1))
                for gi, (p0, gw) in enumerate(DVE_GROUPS):
                    dve.wait_ge(s_sig, r * NACT + gi + 1)
                    dve.tensor_scalar(
                        t255[:, p0 : p0 + gw], sig[r % 2][:, p0 : p0 + gw],
                        255.0, None, mybir.AluOpType.mult,
                    )
                    dve.scalar_tensor_tensor(
                        adj[r % 2][:, p0 : p0 + gw], t255[:, p0 : p0 + gw],
                        25.5, t255[:, p0 : p0 + gw],
                        mybir.AluOpType.is_ge, mybir.AluOpType.mult,
                    ).then_inc(s_adj, 1)

        @block.gpsimd
        def _(gp):
            if probe in ('dma_in', 'pe', 'act', 'noout'):
                return
            for r in range(reps):
                if probe is None:
                    gp.wait_ge(s_adj, r * NDVE + OUT_SPLIT_DVE)
                gp.dma_start(
                    out=out[:, 0:OUT_SPLIT_P], in_=adj[r % 2][:, 0:OUT_SPLIT_P]
                ).then_inc(s_out, 16)
                if probe is None:
                    gp.wait_ge(s_adj, r * NDVE + NDVE)
                gp.dma_start(
                    out=out[:, OUT_SPLIT_P:PACKED_W],
                    in_=adj[r % 2][:, OUT_SPLIT_P:PACKED_W],
                ).then_inc(s_out, 16)
            gp.wait_ge(s_out, 16 * reps * N_OUT_DMA)

    return nc


_CACHED_NC = None


def make_in_maps(x, mats, head_w, head_b):
    x = np.ascontiguousarray(x, dtype=np.float32)
    mats = np.ascontiguousarray(mats, dtype=np.float32)
    head_w = np.asarray(head_w, dtype=np.float32)
    head_b = np.asarray(head_b, dtype=np.float32)

    # contract the task head into the mats (linearity; same as reference)
    w = np.einsum('m,mtd->td', head_w, mats)  # [T, D] f32
    wT = np.ascontiguousarray(w.T).astype(ml_dtypes.bfloat16)  # [D, T]
    biasd = np.full((D, 1), head_b, dtype=np.float32)

    xT = np.ascontiguousarray(x.T).astype(ml_dtypes.float8_e3m4)  # [D, N]

    return [
        {
            "xq": np.ascontiguousarray(xT[:, c * NSH : (c + 1) * NSH]),
            "wT": wT,
            "biasd": biasd,
        }
        for c in range(N_CORES)
    ]


def unpack_out(results):
    out = np.empty((T, N), dtype=np.float32)
    for c in range(N_CORES):
        packed = results[c]["out"].astype(np.float32) * (1.0 / 255.0)
        base = c * NSH
        for i, bw in enumerate(BLOCKS):
            p0 = BLOCK_P0[i]
            q = 0
            for s in subtiles(bw):
                x0 = base + 2 * (p0 + q)
                out[:, x0 : x0 + s] = packed[0:T, p0 + q : p0 + q + s]
                out[:, x0 + s : x0 + 2 * s] = packed[T:D, p0 + q : p0 + q + s]
                q += s
    return out


def kernel(x, mats, head_w, head_b):
    global _CACHED_NC
    if _CACHED_NC is None:
        _CACHED_NC = build_nc()
    nc = _CACHED_NC

    in_maps = make_in_maps(x, mats, head_w, head_b)
    results = run_bass_kernel_spmd(nc, in_maps, core_ids=list(range(N_CORES))).results
    return unpack_out(results)
